# revision 75
# baseline (speedup 1.0000x reference)
"""GCN (GCNConv + ReLU + Linear) Trainium2 kernel, 8-core SPMD.

Strategy (per core, owning a 12500-node dst range):
  - Host packs a padded, dst-sorted edge stream: pairs of stream
    partitions map to one of 64 "slots"; a window = 64 dst nodes; dst
    nodes are assigned to windows sorted by degree so each window's
    batch count ~= its mean (few % padding).  Stream values are
    x[src] * dinv[src] * dinv[dst] in bf16 so the device-side segment
    sum needs no further normalization.
  - Device scatter: matmul with the CONSTANT pair->slot one-hot as the
    stationary operand (amortizing LDWEIGHTS) and the edge stream as
    the moving operand, 32 windows fused per matmul (rhs [128, 448]),
    accumulating agg[slot, (win,f)] in fp32 PSUM over the group's
    batches.  ~110 matmuls total instead of one per 128 edges.
  - Tail per 8-window chunk: PE-transpose agg chunk -> [112, 64],
    append a ones-row, then one matmul against a block-diagonal
    [113, 512] W1-with-b1 constant -> h[slot, (win,h)]; relu (Scalar).
    |W2| is folded into that constant pre-relu and h columns are
    ordered by sign(W2), so y = reduce(pos cols) - reduce(neg cols)
    via two strided DVE segment reduces.  b2 is added on the host.
  - Host un-permutes the degree-sorted output order.
"""
import numpy as np

N = 100000
NE = 3200000
F = 14
H = 64
NC = 8
OWN = N // NC       # 12500
W = 64              # dst slots per window
NWIN = -(-OWN // W)  # 196 windows per core
GW = 32             # windows per scatter group (fused matmul)
CW = 8              # windows per tail chunk
NCHUNK = -(-NWIN // CW)  # 25


def _ranks(keys_sorted):
    """rank of each element within its (already grouped) run."""
    n = len(keys_sorted)
    if n == 0:
        return np.zeros(0, dtype=np.int64)
    change = np.ones(n, dtype=bool)
    change[1:] = keys_sorted[1:] != keys_sorted[:-1]
    run_start = np.maximum.accumulate(np.where(change, np.arange(n), 0))
    return np.arange(n) - run_start


def _host_pack(x, edge_index):
    src = np.concatenate([edge_index[0].astype(np.int64),
                          np.arange(N, dtype=np.int64)])
    dst = np.concatenate([edge_index[1].astype(np.int64),
                          np.arange(N, dtype=np.int64)])
    deg = np.bincount(dst, minlength=N).astype(np.float32)
    dinv = 1.0 / np.sqrt(np.maximum(deg, 1.0))

    # degree-sorted rank of each dst within its core; shared window batch
    # counts B_w = max over cores (program must be uniform across cores)
    rank = np.empty(N, dtype=np.int64)
    orders = []
    bw_pc = np.zeros((NC, NWIN), dtype=np.int64)
    for c in range(NC):
        dc = deg[c * OWN:(c + 1) * OWN]
        o = np.argsort(-dc, kind="stable")
        orders.append(o)
        rank[c * OWN + o] = np.arange(OWN)
        pairs = np.zeros(NWIN * W, dtype=np.int64)
        pairs[:OWN] = (dc[o].astype(np.int64) + 1) // 2
        bw_pc[c] = pairs.reshape(NWIN, W).max(axis=1)
    B_w = np.maximum(bw_pc.max(axis=0), 1)

    # scatter groups of GW windows, padded to the group's max batches
    ngrp = -(-NWIN // GW)
    nw_g = np.array([min(GW, NWIN - g * GW) for g in range(ngrp)])
    B_g = np.array([int(B_w[g * GW:g * GW + nw_g[g]].max())
                    for g in range(ngrp)])
    gbase = np.concatenate([[0], np.cumsum(B_g * nw_g * F)])

    # per-edge placement: sort by dst, rank within dst run
    es = np.argsort(dst, kind="stable")
    dsts = dst[es]
    srcs = src[es]
    r = _ranks(dsts)
    c_e = dsts // OWN
    rk = rank[dsts]
    w_e = rk // W                       # window
    g_e = w_e // GW                     # scatter group
    wl_e = w_e % GW                     # window within group
    p_e = 2 * (rk % W) + (r % 2)        # stream partition (pair slot)
    col_e = gbase[g_e] + (r // 2) * (nw_g[g_e] * F) + wl_e * F

    xs = x * dinv[:, None]
    vals = xs[srcs] * dinv[dsts][:, None]           # [E+N, F] fp32
    totcols = int(gbase[-1])
    stream = np.zeros((NC, 128, totcols), dtype=np.float32)
    stream[c_e[:, None], p_e[:, None],
           col_e[:, None] + np.arange(F)[None, :]] = vals
    stream = _to_bf16(stream)
    spec = tuple(zip(map(int, nw_g), map(int, B_g)))
    return stream, spec, orders


def _build_program(spec, npos):
    import concourse.bass as bass
    import concourse.mybir as mybir
    from concourse import bacc
    from concourse.tile import TileContext

    totcols = sum(nw * bg * F for nw, bg in spec)
    sbtmax = max(bg * nw * F for nw, bg in spec)

    nc = bacc.Bacc("TRN2", target_bir_lowering=False, debug=False,
                   num_devices=NC)
    dt = mybir.dt

    stream = nc.dram_tensor("stream", [128, totcols], dt.bfloat16,
                            kind="ExternalInput")
    # consts blob: [0:64]=pair, [64:128]=ident, [128:640]=w1b —
    # one DMA instead of several
    consts = nc.dram_tensor("consts", [128, 640], dt.bfloat16,
                            kind="ExternalInput")
    yout = nc.dram_tensor("yout", [W, NCHUNK * CW], dt.bfloat16,
                          kind="ExternalOutput")

    with TileContext(nc) as tc:
        with (
            tc.tile_pool(name="persist", bufs=1) as pp,
            tc.tile_pool(name="stream", bufs=16) as sp,
            tc.tile_pool(name="work", bufs=4) as wp,
            tc.tile_pool(name="psum", bufs=2, space="PSUM") as psp,
            tc.tile_pool(name="psum_t", bufs=3, space="PSUM") as pst,
        ):
            cb = pp.tile([128, 640], dt.bfloat16)
            # pair+ident first: the scatter only needs cols 0:128, so
            # the first matmul isn't gated on the w1b transfer
            nc.scalar.dma_start(cb[:, 0:128], consts[:, 0:128])
            nc.scalar.dma_start(cb[:, 128:], consts[:, 128:])
            pair_sb = cb[:, 0:W]
            id_sb = cb[0:W, W:2 * W]
            w1b_sb = cb[:, 128:128 + CW * H]
            y_all = pp.tile([W, NCHUNK * CW], dt.bfloat16)
            # transposed-agg staging tiles; row 112 = constant 1.0
            # (multiplies the b1 row of the block-diagonal W1)
            NSTG = 8
            aggts = [pp.tile([128, W], dt.bfloat16, name=f"aggts{i}")
                     for i in range(NSTG)]
            for t in aggts:
                nc.vector.memset(t[:], 1.0)

            def tail(g, nw, pgrp):
                nchu = -(-nw // CW)
                # phase 1: psum->sbuf copies (paired chunks, fewer ACT
                # fixed costs) + PE transposes
                stage = []
                agg_sb = None
                for lc in range(nchu):
                    c = g * (GW // CW) + lc
                    if lc % 2 == 0:
                        wid = min(2, nchu - lc) * CW * F
                        agg_sb = wp.tile([W, 2 * CW * F], dt.bfloat16,
                                         tag="agg")
                        nc.scalar.activation(
                            agg_sb[:, 0:wid],
                            pgrp[0:W, lc * CW * F:lc * CW * F + wid],
                            mybir.ActivationFunctionType.Copy,
                        )
                    asl = agg_sb[:, (lc % 2) * CW * F:(lc % 2 + 1) * CW * F]
                    aggt_ps = pst.tile([CW * F, W], dt.bfloat16, tag="aggt")
                    nc.tensor.transpose(aggt_ps[:], asl, id_sb[:])
                    aggt = aggts[c % NSTG]
                    nc.scalar.activation(
                        aggt[0:CW * F, :], aggt_ps[:],
                        mybir.ActivationFunctionType.Copy,
                    )
                    stage.append((c, aggt))
                # phase 2: W1-with-b1 matmul, relu, W2 mult+reduce
                for c, aggt in stage:
                    ph = pst.tile([H, CW * H], dt.float32, tag="ph")
                    nc.tensor.matmul(
                        out=ph[:], lhsT=aggt[0:CW * F + 1, :],
                        rhs=w1b_sb[0:CW * F + 1, :],
                        start=True, stop=True,
                    )
                    hb = wp.tile([H, CW * H], dt.bfloat16, tag="hb")
                    nc.scalar.activation(
                        hb[:], ph[:], mybir.ActivationFunctionType.Relu,
                    )
                    # |W2| is folded into w1b pre-relu; h columns are
                    # ordered positives-then-negatives, so y is the
                    # difference of two strided segment reduces
                    h3 = hb[:].rearrange("p (w h) -> p w h", h=H)
                    if 0 < npos < H:
                        yp = wp.tile([H, CW], dt.float32, tag="yp")
                        nc.vector.tensor_reduce(
                            out=yp[:], in_=h3[:, :, 0:npos],
                            axis=mybir.AxisListType.X,
                            op=mybir.AluOpType.add,
                        )
                        yn = wp.tile([H, CW], dt.float32, tag="yn")
                        nc.vector.tensor_reduce(
                            out=yn[:], in_=h3[:, :, npos:H],
                            axis=mybir.AxisListType.X,
                            op=mybir.AluOpType.add,
                        )
                        with nc.allow_low_precision("y output is bf16"):
                            nc.vector.tensor_tensor(
                                out=y_all[:, c * CW:(c + 1) * CW],
                                in0=yp[:], in1=yn[:],
                                op=mybir.AluOpType.subtract,
                            )
                    else:
                        yp = wp.tile([H, CW], dt.float32, tag="yp")
                        nc.vector.tensor_reduce(
                            out=yp[:], in_=h3[:],
                            axis=mybir.AxisListType.X,
                            op=mybir.AluOpType.add,
                        )
                        with nc.allow_low_precision("y output is bf16"):
                            nc.vector.tensor_scalar(
                                out=y_all[:, c * CW:(c + 1) * CW],
                                in0=yp[:],
                                scalar1=1.0 if npos == H else -1.0,
                                scalar2=None,
                                op0=mybir.AluOpType.mult,
                            )

            pending = []
            for g, (nw, bg) in enumerate(spec):
                off = sum(n_ * b_ * F for n_, b_ in spec[:g])
                # one tile per DMA chunk so matmuls only wait for the
                # chunk that covers their batch (dep granularity is
                # per tile, not per slice)
                step = 4
                bounds = list(range(1 if g == 0 else step, bg, step))
                bounds = [0] + bounds + [bg]
                chunks = []
                chunk_of = {}
                for ci in range(len(bounds) - 1):
                    k, hi = bounds[ci], bounds[ci + 1]
                    ct = sp.tile([128, step * GW * F], dt.bfloat16,
                                 tag="sbt")
                    nc.sync.dma_start(
                        ct[:, 0:(hi - k) * nw * F],
                        stream[:, off + k * nw * F:off + hi * nw * F],
                    )
                    chunks.append(ct)
                    for b in range(k, hi):
                        chunk_of[b] = (ct, b - k)
                pgrp = psp.tile([W, GW * F], dt.float32)
                for b in range(bg):
                    ct, lb = chunk_of[b]
                    nc.tensor.matmul(
                        out=pgrp[0:W, 0:nw * F],
                        lhsT=pair_sb[:],
                        rhs=ct[:, lb * nw * F:(lb + 1) * nw * F],
                        start=(b == 0), stop=(b == bg - 1),
                    )
                pending.append((g, nw, pgrp))
                if len(pending) > 1:
                    tail(*pending.pop(0))
                if g == len(spec) - 1:
                    # first part of y is final once earlier tails ran
                    nc.sync.dma_start(yout[:, 0:96], y_all[:, 0:96])
            # all tails except the last group's have been emitted; only
            # the final chunk's 8 columns remain for the closing DMA
            nc.sync.dma_start(yout[:, 96:192], y_all[:, 96:192])
            for p in pending:
                tail(*p)
            nc.sync.dma_start(yout[:, 192:], y_all[:, 192:])

    nc.compile()
    return nc


_CACHE = {}


def kernel(x, edge_index, W1, b1, W2, b2, _want_results_obj=False):
    from concourse import bass_utils

    x = np.asarray(x, dtype=np.float32)
    edge_index = np.asarray(edge_index)
    stream, spec, orders = _host_pack(x, edge_index)

    W1 = np.asarray(W1, dtype=np.float32)
    b1 = np.asarray(b1, dtype=np.float32).reshape(H)
    W2 = np.asarray(W2, dtype=np.float32).reshape(H)
    b2 = float(np.asarray(b2, dtype=np.float32).reshape(()))
    # fold |W2| into W1/b1 (pre-relu scaling); order h columns so
    # positive-W2 entries come first: y = sum(pos) - sum(neg)
    perm = np.argsort(W2 < 0, kind="stable")
    npos = int((W2 >= 0).sum())
    w1s = W1[:, perm] * np.abs(W2[perm])[None, :]
    b1s = b1[perm] * np.abs(W2[perm])

    key = (spec, npos)
    if key not in _CACHE:
        _CACHE[key] = _build_program(spec, npos)
    nc = _CACHE[key]

    consts = np.zeros((128, 640), dtype=np.float32)
    consts[:, 0:W] = np.repeat(np.eye(W, dtype=np.float32), 2, axis=0)
    consts[0:W, W:2 * W] = np.eye(W, dtype=np.float32)
    for w in range(CW):
        consts[w * F:(w + 1) * F, 128 + w * H:128 + (w + 1) * H] = w1s
        consts[CW * F, 128 + w * H:128 + (w + 1) * H] = b1s
    consts = _to_bf16(consts)

    in_maps = []
    for c in range(NC):
        in_maps.append({
            "stream": np.ascontiguousarray(stream[c]),
            "consts": consts,
        })

    try:
        res = bass_utils.run_bass_kernel_spmd(
            nc, in_maps, core_ids=list(range(NC)))
    except Exception:
        # transient NRT device faults recover on re-execution
        res = bass_utils.run_bass_kernel_spmd(
            nc, in_maps, core_ids=list(range(NC)))
    out = np.empty((N, 1), dtype=np.float32)
    for c in range(NC):
        y = np.asarray(res.results[c]["yout"], dtype=np.float32)
        # rank = win*64 + slot  ->  value y[slot, win]
        yr = y[:, :NWIN].T.reshape(-1)[:OWN]  # [win, slot] flat = rank
        out[c * OWN + orders[c], 0] = yr + b2
    if _want_results_obj:
        return out, res
    return out


def _to_bf16(a):
    """fp32 ndarray -> bfloat16 (round-to-nearest-even) as ml_dtypes array."""
    import ml_dtypes

    return a.astype(ml_dtypes.bfloat16)


# revision 77
# speedup vs baseline: 1.0355x; 1.0355x over previous
"""GCN (GCNConv + ReLU + Linear) Trainium2 kernel, 8-core SPMD.

Strategy (per core, owning a 12500-node dst range):
  - Host packs a padded, dst-sorted edge stream: pairs of stream
    partitions map to one of 64 "slots"; a window = 64 dst nodes; dst
    nodes are assigned to windows sorted by degree so each window's
    batch count ~= its mean (few % padding).  Stream values are
    x[src] * dinv[src] * dinv[dst] in bf16 so the device-side segment
    sum needs no further normalization.
  - Device scatter: matmul with the CONSTANT pair->slot one-hot as the
    stationary operand (amortizing LDWEIGHTS) and the edge stream as
    the moving operand, 32 windows fused per matmul (rhs [128, 448]),
    accumulating agg[slot, (win,f)] in fp32 PSUM over the group's
    batches.  ~110 matmuls total instead of one per 128 edges.
  - Tail per 8-window chunk: PE-transpose agg chunk -> [112, 64],
    append a ones-row, then one matmul against a block-diagonal
    [113, 512] W1-with-b1 constant -> h[slot, (win,h)]; relu (Scalar).
    |W2| is folded into that constant pre-relu and h columns are
    ordered by sign(W2), so y = reduce(pos cols) - reduce(neg cols)
    via two strided DVE segment reduces.  b2 is added on the host.
  - Host un-permutes the degree-sorted output order.
"""
import numpy as np

N = 100000
NE = 3200000
F = 14
H = 64
NC = 8
OWN = N // NC       # 12500
W = 64              # dst slots per window
NWIN = -(-OWN // W)  # 196 windows per core
GW = 32             # windows per scatter group (fused matmul)
CW = 8              # windows per tail chunk
NCHUNK = -(-NWIN // CW)  # 25


def _ranks(keys_sorted):
    """rank of each element within its (already grouped) run."""
    n = len(keys_sorted)
    if n == 0:
        return np.zeros(0, dtype=np.int64)
    change = np.ones(n, dtype=bool)
    change[1:] = keys_sorted[1:] != keys_sorted[:-1]
    run_start = np.maximum.accumulate(np.where(change, np.arange(n), 0))
    return np.arange(n) - run_start


def _host_pack(x, edge_index):
    src = np.concatenate([edge_index[0].astype(np.int64),
                          np.arange(N, dtype=np.int64)])
    dst = np.concatenate([edge_index[1].astype(np.int64),
                          np.arange(N, dtype=np.int64)])
    deg = np.bincount(dst, minlength=N).astype(np.float32)
    dinv = 1.0 / np.sqrt(np.maximum(deg, 1.0))

    # degree-sorted rank of each dst within its core; shared window batch
    # counts B_w = max over cores (program must be uniform across cores)
    rank = np.empty(N, dtype=np.int64)
    orders = []
    bw_pc = np.zeros((NC, NWIN), dtype=np.int64)
    for c in range(NC):
        dc = deg[c * OWN:(c + 1) * OWN]
        o = np.argsort(-dc, kind="stable")
        orders.append(o)
        rank[c * OWN + o] = np.arange(OWN)
        pairs = np.zeros(NWIN * W, dtype=np.int64)
        pairs[:OWN] = (dc[o].astype(np.int64) + 1) // 2
        bw_pc[c] = pairs.reshape(NWIN, W).max(axis=1)
    B_w = np.maximum(bw_pc.max(axis=0), 1)

    # scatter groups of GW windows, padded to the group's max batches
    ngrp = -(-NWIN // GW)
    nw_g = np.array([min(GW, NWIN - g * GW) for g in range(ngrp)])
    B_g = np.array([int(B_w[g * GW:g * GW + nw_g[g]].max())
                    for g in range(ngrp)])
    gbase = np.concatenate([[0], np.cumsum(B_g * nw_g * F)])

    # per-edge placement: sort by dst, rank within dst run
    es = np.argsort(dst, kind="stable")
    dsts = dst[es]
    srcs = src[es]
    r = _ranks(dsts)
    c_e = dsts // OWN
    rk = rank[dsts]
    w_e = rk // W                       # window
    g_e = w_e // GW                     # scatter group
    wl_e = w_e % GW                     # window within group
    p_e = 2 * (rk % W) + (r % 2)        # stream partition (pair slot)
    col_e = gbase[g_e] + (r // 2) * (nw_g[g_e] * F) + wl_e * F

    xs = x * dinv[:, None]
    vals = xs[srcs] * dinv[dsts][:, None]           # [E+N, F] fp32
    totcols = int(gbase[-1])
    stream = np.zeros((NC, 128, totcols), dtype=np.float32)
    stream[c_e[:, None], p_e[:, None],
           col_e[:, None] + np.arange(F)[None, :]] = vals
    stream = _to_bf16(stream)
    spec = tuple(zip(map(int, nw_g), map(int, B_g)))
    return stream, spec, orders


def _build_program(spec, npos):
    import concourse.bass as bass
    import concourse.mybir as mybir
    from concourse import bacc
    from concourse.tile import TileContext

    totcols = sum(nw * bg * F for nw, bg in spec)
    sbtmax = max(bg * nw * F for nw, bg in spec)

    nc = bacc.Bacc("TRN2", target_bir_lowering=False, debug=False,
                   num_devices=NC)
    dt = mybir.dt

    stream = nc.dram_tensor("stream", [128, totcols], dt.bfloat16,
                            kind="ExternalInput")
    # consts blob: [0:64]=pair, [64:128]=ident, [128:640]=w1b —
    # one DMA instead of several
    consts = nc.dram_tensor("consts", [128, 640], dt.bfloat16,
                            kind="ExternalInput")
    yout = nc.dram_tensor("yout", [W, NCHUNK * CW], dt.bfloat16,
                          kind="ExternalOutput")

    with TileContext(nc) as tc:
        with (
            tc.tile_pool(name="persist", bufs=1) as pp,
            tc.tile_pool(name="stream", bufs=16) as sp,
            tc.tile_pool(name="work", bufs=4) as wp,
            tc.tile_pool(name="psum", bufs=2, space="PSUM") as psp,
            tc.tile_pool(name="psum_t", bufs=3, space="PSUM") as pst,
        ):
            cb = pp.tile([128, 640], dt.bfloat16)
            nc.scalar.dma_start(cb[:], consts[:])
            pair_sb = cb[:, 0:W]
            id_sb = cb[0:W, W:2 * W]
            w1b_sb = cb[:, 128:128 + CW * H]
            y_all = pp.tile([W, NCHUNK * CW], dt.bfloat16)
            # transposed-agg staging tiles; row 112 = constant 1.0
            # (multiplies the b1 row of the block-diagonal W1)
            NSTG = 8
            aggts = [pp.tile([128, W], dt.bfloat16, name=f"aggts{i}")
                     for i in range(NSTG)]
            for t in aggts:
                nc.vector.memset(t[:], 1.0)

            def tail(g, nw, pgrp):
                nchu = -(-nw // CW)
                # phase 1: psum->sbuf copies (paired chunks, fewer ACT
                # fixed costs) + PE transposes
                stage = []
                agg_sb = None
                for lc in range(nchu):
                    c = g * (GW // CW) + lc
                    if lc % 2 == 0:
                        wid = min(2, nchu - lc) * CW * F
                        agg_sb = wp.tile([W, 2 * CW * F], dt.bfloat16,
                                         tag="agg")
                        nc.scalar.activation(
                            agg_sb[:, 0:wid],
                            pgrp[0:W, lc * CW * F:lc * CW * F + wid],
                            mybir.ActivationFunctionType.Copy,
                        )
                    asl = agg_sb[:, (lc % 2) * CW * F:(lc % 2 + 1) * CW * F]
                    aggt_ps = pst.tile([CW * F, W], dt.bfloat16, tag="aggt")
                    nc.tensor.transpose(aggt_ps[:], asl, id_sb[:])
                    aggt = aggts[c % NSTG]
                    nc.scalar.activation(
                        aggt[0:CW * F, :], aggt_ps[:],
                        mybir.ActivationFunctionType.Copy,
                    )
                    stage.append((c, aggt))
                # phase 2: W1-with-b1 matmul, relu, W2 mult+reduce
                for c, aggt in stage:
                    ph = pst.tile([H, CW * H], dt.float32, tag="ph")
                    nc.tensor.matmul(
                        out=ph[:], lhsT=aggt[0:CW * F + 1, :],
                        rhs=w1b_sb[0:CW * F + 1, :],
                        start=True, stop=True,
                    )
                    hb = wp.tile([H, CW * H], dt.bfloat16, tag="hb")
                    nc.scalar.activation(
                        hb[:], ph[:], mybir.ActivationFunctionType.Relu,
                    )
                    # |W2| is folded into w1b pre-relu; h columns are
                    # ordered positives-then-negatives, so y is the
                    # difference of two strided segment reduces
                    h3 = hb[:].rearrange("p (w h) -> p w h", h=H)
                    if 0 < npos < H:
                        yp = wp.tile([H, CW], dt.float32, tag="yp")
                        nc.vector.tensor_reduce(
                            out=yp[:], in_=h3[:, :, 0:npos],
                            axis=mybir.AxisListType.X,
                            op=mybir.AluOpType.add,
                        )
                        yn = wp.tile([H, CW], dt.float32, tag="yn")
                        nc.vector.tensor_reduce(
                            out=yn[:], in_=h3[:, :, npos:H],
                            axis=mybir.AxisListType.X,
                            op=mybir.AluOpType.add,
                        )
                        with nc.allow_low_precision("y output is bf16"):
                            nc.vector.tensor_tensor(
                                out=y_all[:, c * CW:(c + 1) * CW],
                                in0=yp[:], in1=yn[:],
                                op=mybir.AluOpType.subtract,
                            )
                    else:
                        yp = wp.tile([H, CW], dt.float32, tag="yp")
                        nc.vector.tensor_reduce(
                            out=yp[:], in_=h3[:],
                            axis=mybir.AxisListType.X,
                            op=mybir.AluOpType.add,
                        )
                        with nc.allow_low_precision("y output is bf16"):
                            nc.vector.tensor_scalar(
                                out=y_all[:, c * CW:(c + 1) * CW],
                                in0=yp[:],
                                scalar1=1.0 if npos == H else -1.0,
                                scalar2=None,
                                op0=mybir.AluOpType.mult,
                            )

            pending = []
            for g, (nw, bg) in enumerate(spec):
                off = sum(n_ * b_ * F for n_, b_ in spec[:g])
                # one tile per DMA chunk so matmuls only wait for the
                # chunk that covers their batch (dep granularity is
                # per tile, not per slice)
                step = 4
                bounds = list(range(1 if g == 0 else step, bg, step))
                bounds = [0] + bounds + [bg]
                chunks = []
                chunk_of = {}
                for ci in range(len(bounds) - 1):
                    k, hi = bounds[ci], bounds[ci + 1]
                    ct = sp.tile([128, step * GW * F], dt.bfloat16,
                                 tag="sbt")
                    nc.sync.dma_start(
                        ct[:, 0:(hi - k) * nw * F],
                        stream[:, off + k * nw * F:off + hi * nw * F],
                    )
                    chunks.append(ct)
                    for b in range(k, hi):
                        chunk_of[b] = (ct, b - k)
                pgrp = psp.tile([W, GW * F], dt.float32)
                for b in range(bg):
                    ct, lb = chunk_of[b]
                    nc.tensor.matmul(
                        out=pgrp[0:W, 0:nw * F],
                        lhsT=pair_sb[:],
                        rhs=ct[:, lb * nw * F:(lb + 1) * nw * F],
                        start=(b == 0), stop=(b == bg - 1),
                    )
                pending.append((g, nw, pgrp))
                if len(pending) > 1:
                    tail(*pending.pop(0))
                if g == len(spec) - 1:
                    # first part of y is final once earlier tails ran
                    nc.sync.dma_start(yout[:, 0:96], y_all[:, 0:96])
            for p in pending:
                tail(*p)
            nc.sync.dma_start(yout[:, 96:], y_all[:, 96:])

    nc.compile()
    return nc


_CACHE = {}


def kernel(x, edge_index, W1, b1, W2, b2, _want_results_obj=False):
    from concourse import bass_utils

    x = np.asarray(x, dtype=np.float32)
    edge_index = np.asarray(edge_index)
    stream, spec, orders = _host_pack(x, edge_index)

    W1 = np.asarray(W1, dtype=np.float32)
    b1 = np.asarray(b1, dtype=np.float32).reshape(H)
    W2 = np.asarray(W2, dtype=np.float32).reshape(H)
    b2 = float(np.asarray(b2, dtype=np.float32).reshape(()))
    # fold |W2| into W1/b1 (pre-relu scaling); order h columns so
    # positive-W2 entries come first: y = sum(pos) - sum(neg)
    perm = np.argsort(W2 < 0, kind="stable")
    npos = int((W2 >= 0).sum())
    w1s = W1[:, perm] * np.abs(W2[perm])[None, :]
    b1s = b1[perm] * np.abs(W2[perm])

    key = (spec, npos)
    if key not in _CACHE:
        _CACHE[key] = _build_program(spec, npos)
    nc = _CACHE[key]

    consts = np.zeros((128, 640), dtype=np.float32)
    consts[:, 0:W] = np.repeat(np.eye(W, dtype=np.float32), 2, axis=0)
    consts[0:W, W:2 * W] = np.eye(W, dtype=np.float32)
    for w in range(CW):
        consts[w * F:(w + 1) * F, 128 + w * H:128 + (w + 1) * H] = w1s
        consts[CW * F, 128 + w * H:128 + (w + 1) * H] = b1s
    consts = _to_bf16(consts)

    in_maps = []
    for c in range(NC):
        in_maps.append({
            "stream": np.ascontiguousarray(stream[c]),
            "consts": consts,
        })

    try:
        res = bass_utils.run_bass_kernel_spmd(
            nc, in_maps, core_ids=list(range(NC)))
    except Exception:
        # transient NRT device faults recover on re-execution
        res = bass_utils.run_bass_kernel_spmd(
            nc, in_maps, core_ids=list(range(NC)))
    out = np.empty((N, 1), dtype=np.float32)
    for c in range(NC):
        y = np.asarray(res.results[c]["yout"], dtype=np.float32)
        # rank = win*64 + slot  ->  value y[slot, win]
        yr = y[:, :NWIN].T.reshape(-1)[:OWN]  # [win, slot] flat = rank
        out[c * OWN + orders[c], 0] = yr + b2
    if _want_results_obj:
        return out, res
    return out


def _to_bf16(a):
    """fp32 ndarray -> bfloat16 (round-to-nearest-even) as ml_dtypes array."""
    import ml_dtypes

    return a.astype(ml_dtypes.bfloat16)


# revision 80
# speedup vs baseline: 1.0447x; 1.0089x over previous
"""GCN (GCNConv + ReLU + Linear) Trainium2 kernel, 8-core SPMD.

Strategy (per core, owning a 12500-node dst range):
  - Host packs a padded, dst-sorted edge stream: pairs of stream
    partitions map to one of 64 "slots"; a window = 64 dst nodes; dst
    nodes are assigned to windows sorted by degree so each window's
    batch count ~= its mean (few % padding).  Stream values are
    x[src] * dinv[src] * dinv[dst] in bf16 so the device-side segment
    sum needs no further normalization.
  - Device scatter: matmul with the CONSTANT pair->slot one-hot as the
    stationary operand (amortizing LDWEIGHTS) and the edge stream as
    the moving operand, 32 windows fused per matmul (rhs [128, 448]),
    accumulating agg[slot, (win,f)] in fp32 PSUM over the group's
    batches.  ~110 matmuls total instead of one per 128 edges.
  - Tail per 8-window chunk: PE-transpose agg chunk -> [112, 64],
    append a ones-row, then one matmul against a block-diagonal
    [113, 512] W1-with-b1 constant -> h[slot, (win,h)]; relu (Scalar).
    |W2| is folded into that constant pre-relu and h columns are
    ordered by sign(W2), so y = reduce(pos cols) - reduce(neg cols)
    via two strided DVE segment reduces.  b2 is added on the host.
  - Host un-permutes the degree-sorted output order.
"""
import numpy as np

N = 100000
NE = 3200000
F = 14
H = 64
NC = 8
OWN = N // NC       # 12500
W = 64              # dst slots per window
NWIN = -(-OWN // W)  # 196 windows per core
GW = 32             # windows per scatter group (fused matmul)
CW = 8              # windows per tail chunk
NCHUNK = -(-NWIN // CW)  # 25


def _ranks(keys_sorted):
    """rank of each element within its (already grouped) run."""
    n = len(keys_sorted)
    if n == 0:
        return np.zeros(0, dtype=np.int64)
    change = np.ones(n, dtype=bool)
    change[1:] = keys_sorted[1:] != keys_sorted[:-1]
    run_start = np.maximum.accumulate(np.where(change, np.arange(n), 0))
    return np.arange(n) - run_start


def _host_pack(x, edge_index):
    src = np.concatenate([edge_index[0].astype(np.int64),
                          np.arange(N, dtype=np.int64)])
    dst = np.concatenate([edge_index[1].astype(np.int64),
                          np.arange(N, dtype=np.int64)])
    deg = np.bincount(dst, minlength=N).astype(np.float32)
    dinv = 1.0 / np.sqrt(np.maximum(deg, 1.0))

    # degree-sorted rank of each dst within its core; shared window batch
    # counts B_w = max over cores (program must be uniform across cores)
    rank = np.empty(N, dtype=np.int64)
    orders = []
    bw_pc = np.zeros((NC, NWIN), dtype=np.int64)
    for c in range(NC):
        dc = deg[c * OWN:(c + 1) * OWN]
        o = np.argsort(-dc, kind="stable")
        orders.append(o)
        rank[c * OWN + o] = np.arange(OWN)
        pairs = np.zeros(NWIN * W, dtype=np.int64)
        pairs[:OWN] = (dc[o].astype(np.int64) + 1) // 2
        bw_pc[c] = pairs.reshape(NWIN, W).max(axis=1)
    B_w = np.maximum(bw_pc.max(axis=0), 1)

    # scatter groups of GW windows, padded to the group's max batches
    ngrp = -(-NWIN // GW)
    nw_g = np.array([min(GW, NWIN - g * GW) for g in range(ngrp)])
    B_g = np.array([int(B_w[g * GW:g * GW + nw_g[g]].max())
                    for g in range(ngrp)])
    gbase = np.concatenate([[0], np.cumsum(B_g * nw_g * F)])

    # per-edge placement: sort by dst, rank within dst run
    es = np.argsort(dst, kind="stable")
    dsts = dst[es]
    srcs = src[es]
    r = _ranks(dsts)
    c_e = dsts // OWN
    rk = rank[dsts]
    w_e = rk // W                       # window
    g_e = w_e // GW                     # scatter group
    wl_e = w_e % GW                     # window within group
    p_e = 2 * (rk % W) + (r % 2)        # stream partition (pair slot)
    col_e = gbase[g_e] + (r // 2) * (nw_g[g_e] * F) + wl_e * F

    xs = x * dinv[:, None]
    vals = xs[srcs] * dinv[dsts][:, None]           # [E+N, F] fp32
    totcols = int(gbase[-1])
    stream = np.zeros((NC, 128, totcols), dtype=np.float32)
    stream[c_e[:, None], p_e[:, None],
           col_e[:, None] + np.arange(F)[None, :]] = vals
    stream = _to_bf16(stream)
    spec = tuple(zip(map(int, nw_g), map(int, B_g)))
    return stream, spec, orders


def _build_program(spec, npos):
    import concourse.bass as bass
    import concourse.mybir as mybir
    from concourse import bacc
    from concourse.tile import TileContext

    totcols = sum(nw * bg * F for nw, bg in spec)
    sbtmax = max(bg * nw * F for nw, bg in spec)

    nc = bacc.Bacc("TRN2", target_bir_lowering=False, debug=False,
                   num_devices=NC)
    dt = mybir.dt

    stream = nc.dram_tensor("stream", [128, totcols], dt.bfloat16,
                            kind="ExternalInput")
    # consts blob: [0:64]=pair, [64:128]=ident, [128:640]=w1b —
    # one DMA instead of several
    consts = nc.dram_tensor("consts", [128, 640], dt.bfloat16,
                            kind="ExternalInput")
    yout = nc.dram_tensor("yout", [W, NCHUNK * CW], dt.bfloat16,
                          kind="ExternalOutput")

    with TileContext(nc) as tc:
        with (
            tc.tile_pool(name="persist", bufs=1) as pp,
            tc.tile_pool(name="stream", bufs=16) as sp,
            tc.tile_pool(name="work", bufs=4) as wp,
            tc.tile_pool(name="psum", bufs=2, space="PSUM") as psp,
            tc.tile_pool(name="psum_t", bufs=3, space="PSUM") as pst,
        ):
            cb = pp.tile([128, 640], dt.bfloat16)
            nc.scalar.dma_start(cb[:], consts[:])
            pair_sb = cb[:, 0:W]
            id_sb = cb[0:W, W:2 * W]
            w1b_sb = cb[:, 128:128 + CW * H]
            y_all = pp.tile([W, NCHUNK * CW], dt.bfloat16)
            # transposed-agg staging tiles; row 112 = constant 1.0
            # (multiplies the b1 row of the block-diagonal W1)
            NSTG = 4
            aggts = [pp.tile([128, 2 * W], dt.bfloat16, name=f"aggts{i}")
                     for i in range(NSTG)]
            for t in aggts:
                nc.vector.memset(t[:], 1.0)

            def tail(g, nw, pgrp):
                nchu = -(-nw // CW)
                # phase 1: paired psum->sbuf copies + PE transposes.
                # Two chunk transposes land in disjoint column halves
                # of one psum tile (full overwrites, values persist),
                # so one ACT copy stages both — halving the per-op
                # fixed PSUM-read costs on the Scalar engine.
                stage = []
                agg_sb = None
                aggt_ps = None
                for lc in range(nchu):
                    c = g * (GW // CW) + lc
                    if lc % 2 == 0:
                        wid = min(2, nchu - lc) * CW * F
                        agg_sb = wp.tile([W, 2 * CW * F], dt.bfloat16,
                                         tag="agg")
                        nc.scalar.activation(
                            agg_sb[:, 0:wid],
                            pgrp[0:W, lc * CW * F:lc * CW * F + wid],
                            mybir.ActivationFunctionType.Copy,
                        )
                        aggt_ps = pst.tile([CW * F, 2 * W], dt.bfloat16,
                                           tag="aggt")
                    half = lc % 2
                    asl = agg_sb[:, half * CW * F:(half + 1) * CW * F]
                    nc.tensor.transpose(
                        aggt_ps[:, half * W:(half + 1) * W], asl, id_sb[:])
                    if half == 1 or lc == nchu - 1:
                        wid = (half + 1) * W
                        aggt = aggts[(c // 2) % NSTG]
                        nc.scalar.activation(
                            aggt[0:CW * F, 0:wid], aggt_ps[:, 0:wid],
                            mybir.ActivationFunctionType.Copy,
                        )
                        for h2 in range(half + 1):
                            stage.append((c - half + h2, aggt, h2))
                # phase 2: W1-with-b1 matmul, relu, W2 sign-split reduce
                for c, aggt, h2 in stage:
                    ph = pst.tile([H, CW * H], dt.float32, tag="ph")
                    nc.tensor.matmul(
                        out=ph[:],
                        lhsT=aggt[0:CW * F + 1, h2 * W:(h2 + 1) * W],
                        rhs=w1b_sb[0:CW * F + 1, :],
                        start=True, stop=True,
                    )
                    hb = wp.tile([H, CW * H], dt.bfloat16, tag="hb")
                    nc.scalar.activation(
                        hb[:], ph[:], mybir.ActivationFunctionType.Relu,
                    )
                    # |W2| is folded into w1b pre-relu; h columns are
                    # ordered positives-then-negatives, so y is the
                    # difference of two strided segment reduces
                    h3 = hb[:].rearrange("p (w h) -> p w h", h=H)
                    if 0 < npos < H:
                        yp = wp.tile([H, CW], dt.float32, tag="yp")
                        nc.vector.tensor_reduce(
                            out=yp[:], in_=h3[:, :, 0:npos],
                            axis=mybir.AxisListType.X,
                            op=mybir.AluOpType.add,
                        )
                        yn = wp.tile([H, CW], dt.float32, tag="yn")
                        nc.vector.tensor_reduce(
                            out=yn[:], in_=h3[:, :, npos:H],
                            axis=mybir.AxisListType.X,
                            op=mybir.AluOpType.add,
                        )
                        with nc.allow_low_precision("y output is bf16"):
                            nc.vector.tensor_tensor(
                                out=y_all[:, c * CW:(c + 1) * CW],
                                in0=yp[:], in1=yn[:],
                                op=mybir.AluOpType.subtract,
                            )
                    else:
                        yp = wp.tile([H, CW], dt.float32, tag="yp")
                        nc.vector.tensor_reduce(
                            out=yp[:], in_=h3[:],
                            axis=mybir.AxisListType.X,
                            op=mybir.AluOpType.add,
                        )
                        with nc.allow_low_precision("y output is bf16"):
                            nc.vector.tensor_scalar(
                                out=y_all[:, c * CW:(c + 1) * CW],
                                in0=yp[:],
                                scalar1=1.0 if npos == H else -1.0,
                                scalar2=None,
                                op0=mybir.AluOpType.mult,
                            )

            pending = []
            for g, (nw, bg) in enumerate(spec):
                off = sum(n_ * b_ * F for n_, b_ in spec[:g])
                # one tile per DMA chunk so matmuls only wait for the
                # chunk that covers their batch (dep granularity is
                # per tile, not per slice)
                step = 4
                bounds = list(range(1 if g == 0 else step, bg, step))
                bounds = [0] + bounds + [bg]
                chunks = []
                chunk_of = {}
                for ci in range(len(bounds) - 1):
                    k, hi = bounds[ci], bounds[ci + 1]
                    ct = sp.tile([128, step * GW * F], dt.bfloat16,
                                 tag="sbt")
                    nc.sync.dma_start(
                        ct[:, 0:(hi - k) * nw * F],
                        stream[:, off + k * nw * F:off + hi * nw * F],
                    )
                    chunks.append(ct)
                    for b in range(k, hi):
                        chunk_of[b] = (ct, b - k)
                pgrp = psp.tile([W, GW * F], dt.float32)
                for b in range(bg):
                    ct, lb = chunk_of[b]
                    nc.tensor.matmul(
                        out=pgrp[0:W, 0:nw * F],
                        lhsT=pair_sb[:],
                        rhs=ct[:, lb * nw * F:(lb + 1) * nw * F],
                        start=(b == 0), stop=(b == bg - 1),
                    )
                pending.append((g, nw, pgrp))
                if len(pending) > 1:
                    tail(*pending.pop(0))
                if g == len(spec) - 1:
                    # first part of y is final once earlier tails ran
                    nc.sync.dma_start(yout[:, 0:96], y_all[:, 0:96])
            for p in pending:
                tail(*p)
            nc.sync.dma_start(yout[:, 96:], y_all[:, 96:])

    nc.compile()
    return nc


_CACHE = {}


def kernel(x, edge_index, W1, b1, W2, b2, _want_results_obj=False):
    from concourse import bass_utils

    x = np.asarray(x, dtype=np.float32)
    edge_index = np.asarray(edge_index)
    stream, spec, orders = _host_pack(x, edge_index)

    W1 = np.asarray(W1, dtype=np.float32)
    b1 = np.asarray(b1, dtype=np.float32).reshape(H)
    W2 = np.asarray(W2, dtype=np.float32).reshape(H)
    b2 = float(np.asarray(b2, dtype=np.float32).reshape(()))
    # fold |W2| into W1/b1 (pre-relu scaling); order h columns so
    # positive-W2 entries come first: y = sum(pos) - sum(neg)
    perm = np.argsort(W2 < 0, kind="stable")
    npos = int((W2 >= 0).sum())
    w1s = W1[:, perm] * np.abs(W2[perm])[None, :]
    b1s = b1[perm] * np.abs(W2[perm])

    key = (spec, npos)
    if key not in _CACHE:
        _CACHE[key] = _build_program(spec, npos)
    nc = _CACHE[key]

    consts = np.zeros((128, 640), dtype=np.float32)
    consts[:, 0:W] = np.repeat(np.eye(W, dtype=np.float32), 2, axis=0)
    consts[0:W, W:2 * W] = np.eye(W, dtype=np.float32)
    for w in range(CW):
        consts[w * F:(w + 1) * F, 128 + w * H:128 + (w + 1) * H] = w1s
        consts[CW * F, 128 + w * H:128 + (w + 1) * H] = b1s
    consts = _to_bf16(consts)

    in_maps = []
    for c in range(NC):
        in_maps.append({
            "stream": np.ascontiguousarray(stream[c]),
            "consts": consts,
        })

    try:
        res = bass_utils.run_bass_kernel_spmd(
            nc, in_maps, core_ids=list(range(NC)))
    except Exception:
        # transient NRT device faults recover on re-execution
        res = bass_utils.run_bass_kernel_spmd(
            nc, in_maps, core_ids=list(range(NC)))
    out = np.empty((N, 1), dtype=np.float32)
    for c in range(NC):
        y = np.asarray(res.results[c]["yout"], dtype=np.float32)
        # rank = win*64 + slot  ->  value y[slot, win]
        yr = y[:, :NWIN].T.reshape(-1)[:OWN]  # [win, slot] flat = rank
        out[c * OWN + orders[c], 0] = yr + b2
    if _want_results_obj:
        return out, res
    return out


def _to_bf16(a):
    """fp32 ndarray -> bfloat16 (round-to-nearest-even) as ml_dtypes array."""
    import ml_dtypes

    return a.astype(ml_dtypes.bfloat16)


# revision 81
# speedup vs baseline: 1.0615x; 1.0161x over previous
"""GCN (GCNConv + ReLU + Linear) Trainium2 kernel, 8-core SPMD.

Strategy (per core, owning a 12500-node dst range):
  - Host packs a padded, dst-sorted edge stream: pairs of stream
    partitions map to one of 64 "slots"; a window = 64 dst nodes; dst
    nodes are assigned to windows sorted by degree so each window's
    batch count ~= its mean (few % padding).  Stream values are
    x[src] * dinv[src] * dinv[dst] in bf16 so the device-side segment
    sum needs no further normalization.
  - Device scatter: matmul with the CONSTANT pair->slot one-hot as the
    stationary operand (amortizing LDWEIGHTS) and the edge stream as
    the moving operand, 32 windows fused per matmul (rhs [128, 448]),
    accumulating agg[slot, (win,f)] in fp32 PSUM over the group's
    batches.  ~110 matmuls total instead of one per 128 edges.
  - Tail per 8-window chunk: PE-transpose agg chunk -> [112, 64],
    append a ones-row, then one matmul against a block-diagonal
    [113, 512] W1-with-b1 constant -> h[slot, (win,h)]; relu (Scalar).
    |W2| is folded into that constant pre-relu and h columns are
    ordered by sign(W2), so y = reduce(pos cols) - reduce(neg cols)
    via two strided DVE segment reduces.  b2 is added on the host.
  - Host un-permutes the degree-sorted output order.
"""
import numpy as np

N = 100000
NE = 3200000
F = 14
H = 64
NC = 8
OWN = N // NC       # 12500
W = 64              # dst slots per window
NWIN = -(-OWN // W)  # 196 windows per core
GW = 32             # windows per scatter group (fused matmul)
CW = 8              # windows per tail chunk
NCHUNK = -(-NWIN // CW)  # 25


def _ranks(keys_sorted):
    """rank of each element within its (already grouped) run."""
    n = len(keys_sorted)
    if n == 0:
        return np.zeros(0, dtype=np.int64)
    change = np.ones(n, dtype=bool)
    change[1:] = keys_sorted[1:] != keys_sorted[:-1]
    run_start = np.maximum.accumulate(np.where(change, np.arange(n), 0))
    return np.arange(n) - run_start


def _host_pack(x, edge_index):
    src = np.concatenate([edge_index[0].astype(np.int64),
                          np.arange(N, dtype=np.int64)])
    dst = np.concatenate([edge_index[1].astype(np.int64),
                          np.arange(N, dtype=np.int64)])
    deg = np.bincount(dst, minlength=N).astype(np.float32)
    dinv = 1.0 / np.sqrt(np.maximum(deg, 1.0))

    # degree-sorted rank of each dst within its core; shared window batch
    # counts B_w = max over cores (program must be uniform across cores)
    rank = np.empty(N, dtype=np.int64)
    orders = []
    bw_pc = np.zeros((NC, NWIN), dtype=np.int64)
    for c in range(NC):
        dc = deg[c * OWN:(c + 1) * OWN]
        o = np.argsort(-dc, kind="stable")
        orders.append(o)
        rank[c * OWN + o] = np.arange(OWN)
        pairs = np.zeros(NWIN * W, dtype=np.int64)
        pairs[:OWN] = (dc[o].astype(np.int64) + 1) // 2
        bw_pc[c] = pairs.reshape(NWIN, W).max(axis=1)
    B_w = np.maximum(bw_pc.max(axis=0), 1)

    # scatter groups of GW windows, padded to the group's max batches
    ngrp = -(-NWIN // GW)
    nw_g = np.array([min(GW, NWIN - g * GW) for g in range(ngrp)])
    B_g = np.array([int(B_w[g * GW:g * GW + nw_g[g]].max())
                    for g in range(ngrp)])
    gbase = np.concatenate([[0], np.cumsum(B_g * nw_g * F)])

    # per-edge placement: sort by dst, rank within dst run
    es = np.argsort(dst, kind="stable")
    dsts = dst[es]
    srcs = src[es]
    r = _ranks(dsts)
    c_e = dsts // OWN
    rk = rank[dsts]
    w_e = rk // W                       # window
    g_e = w_e // GW                     # scatter group
    wl_e = w_e % GW                     # window within group
    p_e = 2 * (rk % W) + (r % 2)        # stream partition (pair slot)
    col_e = gbase[g_e] + (r // 2) * (nw_g[g_e] * F) + wl_e * F

    xs = x * dinv[:, None]
    vals = xs[srcs] * dinv[dsts][:, None]           # [E+N, F] fp32
    totcols = int(gbase[-1])
    stream = np.zeros((NC, 128, totcols), dtype=np.float32)
    stream[c_e[:, None], p_e[:, None],
           col_e[:, None] + np.arange(F)[None, :]] = vals
    stream = _to_bf16(stream)
    spec = tuple(zip(map(int, nw_g), map(int, B_g)))
    return stream, spec, orders


def _build_program(spec, npos):
    import concourse.bass as bass
    import concourse.mybir as mybir
    from concourse import bacc
    from concourse.tile import TileContext

    totcols = sum(nw * bg * F for nw, bg in spec)
    sbtmax = max(bg * nw * F for nw, bg in spec)

    nc = bacc.Bacc("TRN2", target_bir_lowering=False, debug=False,
                   num_devices=NC)
    dt = mybir.dt

    stream = nc.dram_tensor("stream", [128, totcols], dt.bfloat16,
                            kind="ExternalInput")
    # consts blob: [0:64]=pair, [64:128]=ident, [128:640]=w1b —
    # one DMA instead of several
    consts = nc.dram_tensor("consts", [128, 640], dt.bfloat16,
                            kind="ExternalInput")
    yout = nc.dram_tensor("yout", [W, NCHUNK * CW], dt.bfloat16,
                          kind="ExternalOutput")

    with TileContext(nc) as tc:
        with (
            tc.tile_pool(name="persist", bufs=1) as pp,
            tc.tile_pool(name="stream", bufs=16) as sp,
            tc.tile_pool(name="work", bufs=4) as wp,
            tc.tile_pool(name="psum", bufs=2, space="PSUM") as psp,
            tc.tile_pool(name="psum_t", bufs=3, space="PSUM") as pst,
        ):
            cb = pp.tile([128, 640], dt.bfloat16)
            nc.scalar.dma_start(cb[:], consts[:])
            pair_sb = cb[:, 0:W]
            id_sb = cb[0:W, W:2 * W]
            w1b_sb = cb[:, 128:128 + CW * H]
            y_all = pp.tile([W, NCHUNK * CW], dt.bfloat16)
            # transposed-agg staging tiles; row 112 = constant 1.0
            # (multiplies the b1 row of the block-diagonal W1)
            NSTG = 8
            aggts = [pp.tile([128, W], dt.bfloat16, name=f"aggts{i}")
                     for i in range(NSTG)]
            for t in aggts:
                nc.vector.memset(t[:], 1.0)

            def tail(g, nw, pgrp):
                nchu = -(-nw // CW)
                # phase 1: psum->sbuf copies (paired chunks, fewer ACT
                # fixed costs) + PE transposes
                stage = []
                agg_sb = None
                for lc in range(nchu):
                    c = g * (GW // CW) + lc
                    if lc % 2 == 0:
                        wid = min(2, nchu - lc) * CW * F
                        agg_sb = wp.tile([W, 2 * CW * F], dt.bfloat16,
                                         tag="agg")
                        nc.scalar.activation(
                            agg_sb[:, 0:wid],
                            pgrp[0:W, lc * CW * F:lc * CW * F + wid],
                            mybir.ActivationFunctionType.Copy,
                        )
                    asl = agg_sb[:, (lc % 2) * CW * F:(lc % 2 + 1) * CW * F]
                    aggt_ps = pst.tile([CW * F, W], dt.bfloat16, tag="aggt")
                    nc.tensor.transpose(aggt_ps[:], asl, id_sb[:])
                    aggt = aggts[c % NSTG]
                    nc.scalar.activation(
                        aggt[0:CW * F, :], aggt_ps[:],
                        mybir.ActivationFunctionType.Copy,
                    )
                    stage.append((c, aggt))
                # phase 2: W1-with-b1 matmul, relu, W2 mult+reduce
                for c, aggt in stage:
                    ph = pst.tile([H, CW * H], dt.float32, tag="ph")
                    nc.tensor.matmul(
                        out=ph[:], lhsT=aggt[0:CW * F + 1, :],
                        rhs=w1b_sb[0:CW * F + 1, :],
                        start=True, stop=True,
                    )
                    hb = wp.tile([H, CW * H], dt.bfloat16, tag="hb")
                    nc.scalar.activation(
                        hb[:], ph[:], mybir.ActivationFunctionType.Relu,
                    )
                    # |W2| is folded into w1b pre-relu; h columns are
                    # ordered positives-then-negatives, so y is the
                    # difference of two strided segment reduces
                    h3 = hb[:].rearrange("p (w h) -> p w h", h=H)
                    if 0 < npos < H:
                        yp = wp.tile([H, CW], dt.float32, tag="yp")
                        nc.vector.tensor_reduce(
                            out=yp[:], in_=h3[:, :, 0:npos],
                            axis=mybir.AxisListType.X,
                            op=mybir.AluOpType.add,
                        )
                        yn = wp.tile([H, CW], dt.float32, tag="yn")
                        nc.vector.tensor_reduce(
                            out=yn[:], in_=h3[:, :, npos:H],
                            axis=mybir.AxisListType.X,
                            op=mybir.AluOpType.add,
                        )
                        with nc.allow_low_precision("y output is bf16"):
                            nc.vector.tensor_tensor(
                                out=y_all[:, c * CW:(c + 1) * CW],
                                in0=yp[:], in1=yn[:],
                                op=mybir.AluOpType.subtract,
                            )
                    else:
                        yp = wp.tile([H, CW], dt.float32, tag="yp")
                        nc.vector.tensor_reduce(
                            out=yp[:], in_=h3[:],
                            axis=mybir.AxisListType.X,
                            op=mybir.AluOpType.add,
                        )
                        with nc.allow_low_precision("y output is bf16"):
                            nc.vector.tensor_scalar(
                                out=y_all[:, c * CW:(c + 1) * CW],
                                in0=yp[:],
                                scalar1=1.0 if npos == H else -1.0,
                                scalar2=None,
                                op0=mybir.AluOpType.mult,
                            )

            pending = []
            for g, (nw, bg) in enumerate(spec):
                off = sum(n_ * b_ * F for n_, b_ in spec[:g])
                # one tile per DMA chunk so matmuls only wait for the
                # chunk that covers their batch (dep granularity is
                # per tile, not per slice)
                step = 4
                bounds = list(range(1 if g == 0 else step, bg, step))
                bounds = [0] + bounds + [bg]
                chunks = []
                chunk_of = {}
                for ci in range(len(bounds) - 1):
                    k, hi = bounds[ci], bounds[ci + 1]
                    ct = sp.tile([128, step * GW * F], dt.bfloat16,
                                 tag="sbt")
                    nc.sync.dma_start(
                        ct[:, 0:(hi - k) * nw * F],
                        stream[:, off + k * nw * F:off + hi * nw * F],
                    )
                    chunks.append(ct)
                    for b in range(k, hi):
                        chunk_of[b] = (ct, b - k)
                pgrp = psp.tile([W, GW * F], dt.float32)
                for b in range(bg):
                    ct, lb = chunk_of[b]
                    nc.tensor.matmul(
                        out=pgrp[0:W, 0:nw * F],
                        lhsT=pair_sb[:],
                        rhs=ct[:, lb * nw * F:(lb + 1) * nw * F],
                        start=(b == 0), stop=(b == bg - 1),
                    )
                pending.append((g, nw, pgrp))
                if len(pending) > 1:
                    tail(*pending.pop(0))
                if g == len(spec) - 1:
                    # first part of y is final once earlier tails ran
                    nc.sync.dma_start(yout[:, 0:96], y_all[:, 0:96])
            for p in pending:
                tail(*p)
            nc.sync.dma_start(yout[:, 96:], y_all[:, 96:])

    nc.compile()
    return nc


_CACHE = {}


def kernel(x, edge_index, W1, b1, W2, b2, _want_results_obj=False):
    from concourse import bass_utils

    x = np.asarray(x, dtype=np.float32)
    edge_index = np.asarray(edge_index)
    stream, spec, orders = _host_pack(x, edge_index)

    W1 = np.asarray(W1, dtype=np.float32)
    b1 = np.asarray(b1, dtype=np.float32).reshape(H)
    W2 = np.asarray(W2, dtype=np.float32).reshape(H)
    b2 = float(np.asarray(b2, dtype=np.float32).reshape(()))
    # fold |W2| into W1/b1 (pre-relu scaling); order h columns so
    # positive-W2 entries come first: y = sum(pos) - sum(neg)
    perm = np.argsort(W2 < 0, kind="stable")
    npos = int((W2 >= 0).sum())
    w1s = W1[:, perm] * np.abs(W2[perm])[None, :]
    b1s = b1[perm] * np.abs(W2[perm])

    key = (spec, npos)
    if key not in _CACHE:
        _CACHE[key] = _build_program(spec, npos)
    nc = _CACHE[key]

    consts = np.zeros((128, 640), dtype=np.float32)
    consts[:, 0:W] = np.repeat(np.eye(W, dtype=np.float32), 2, axis=0)
    consts[0:W, W:2 * W] = np.eye(W, dtype=np.float32)
    for w in range(CW):
        consts[w * F:(w + 1) * F, 128 + w * H:128 + (w + 1) * H] = w1s
        consts[CW * F, 128 + w * H:128 + (w + 1) * H] = b1s
    consts = _to_bf16(consts)

    in_maps = []
    for c in range(NC):
        in_maps.append({
            "stream": np.ascontiguousarray(stream[c]),
            "consts": consts,
        })

    try:
        res = bass_utils.run_bass_kernel_spmd(
            nc, in_maps, core_ids=list(range(NC)))
    except Exception:
        # transient NRT device faults recover on re-execution
        res = bass_utils.run_bass_kernel_spmd(
            nc, in_maps, core_ids=list(range(NC)))
    out = np.empty((N, 1), dtype=np.float32)
    for c in range(NC):
        y = np.asarray(res.results[c]["yout"], dtype=np.float32)
        # rank = win*64 + slot  ->  value y[slot, win]
        yr = y[:, :NWIN].T.reshape(-1)[:OWN]  # [win, slot] flat = rank
        out[c * OWN + orders[c], 0] = yr + b2
    if _want_results_obj:
        return out, res
    return out


def _to_bf16(a):
    """fp32 ndarray -> bfloat16 (round-to-nearest-even) as ml_dtypes array."""
    import ml_dtypes

    return a.astype(ml_dtypes.bfloat16)


# revision 84
# speedup vs baseline: 1.0689x; 1.0069x over previous
"""GCN (GCNConv + ReLU + Linear) Trainium2 kernel, 8-core SPMD.

Strategy (per core, owning a 12500-node dst range):
  - Host packs a padded, dst-sorted edge stream: pairs of stream
    partitions map to one of 64 "slots"; a window = 64 dst nodes; dst
    nodes are assigned to windows sorted by degree so each window's
    batch count ~= its mean (few % padding).  Stream values are
    x[src] * dinv[src] * dinv[dst] in bf16 so the device-side segment
    sum needs no further normalization.
  - Device scatter: matmul with the CONSTANT pair->slot one-hot as the
    stationary operand (amortizing LDWEIGHTS) and the edge stream as
    the moving operand, 32 windows fused per matmul (rhs [128, 448]),
    accumulating agg[slot, (win,f)] in fp32 PSUM over the group's
    batches.  ~110 matmuls total instead of one per 128 edges.
  - Tail per 8-window chunk: PE-transpose agg chunk -> [112, 64],
    append a ones-row, then one matmul against a block-diagonal
    [113, 512] W1-with-b1 constant -> h[slot, (win,h)]; relu (Scalar).
    |W2| is folded into that constant pre-relu and h columns are
    ordered by sign(W2), so y = reduce(pos cols) - reduce(neg cols)
    via two strided DVE segment reduces.  b2 is added on the host.
  - Host un-permutes the degree-sorted output order.
"""
import numpy as np

N = 100000
NE = 3200000
F = 14
H = 64
NC = 8
OWN = N // NC       # 12500
W = 64              # dst slots per window
NWIN = -(-OWN // W)  # 196 windows per core
GW = 32             # windows per scatter group (fused matmul)
CW = 8              # windows per tail chunk
NCHUNK = -(-NWIN // CW)  # 25


def _ranks(keys_sorted):
    """rank of each element within its (already grouped) run."""
    n = len(keys_sorted)
    if n == 0:
        return np.zeros(0, dtype=np.int64)
    change = np.ones(n, dtype=bool)
    change[1:] = keys_sorted[1:] != keys_sorted[:-1]
    run_start = np.maximum.accumulate(np.where(change, np.arange(n), 0))
    return np.arange(n) - run_start


def _host_pack(x, edge_index):
    src = np.concatenate([edge_index[0].astype(np.int64),
                          np.arange(N, dtype=np.int64)])
    dst = np.concatenate([edge_index[1].astype(np.int64),
                          np.arange(N, dtype=np.int64)])
    deg = np.bincount(dst, minlength=N).astype(np.float32)
    dinv = 1.0 / np.sqrt(np.maximum(deg, 1.0))

    # degree-sorted rank of each dst within its core; shared window batch
    # counts B_w = max over cores (program must be uniform across cores)
    rank = np.empty(N, dtype=np.int64)
    orders = []
    bw_pc = np.zeros((NC, NWIN), dtype=np.int64)
    for c in range(NC):
        dc = deg[c * OWN:(c + 1) * OWN]
        o = np.argsort(-dc, kind="stable")
        orders.append(o)
        rank[c * OWN + o] = np.arange(OWN)
        pairs = np.zeros(NWIN * W, dtype=np.int64)
        pairs[:OWN] = (dc[o].astype(np.int64) + 1) // 2
        bw_pc[c] = pairs.reshape(NWIN, W).max(axis=1)
    B_w = np.maximum(bw_pc.max(axis=0), 1)

    # scatter groups of GW windows, padded to the group's max batches
    ngrp = -(-NWIN // GW)
    nw_g = np.array([min(GW, NWIN - g * GW) for g in range(ngrp)])
    B_g = np.array([int(B_w[g * GW:g * GW + nw_g[g]].max())
                    for g in range(ngrp)])
    gbase = np.concatenate([[0], np.cumsum(B_g * nw_g * F)])

    # per-edge placement: sort by dst, rank within dst run
    es = np.argsort(dst, kind="stable")
    dsts = dst[es]
    srcs = src[es]
    r = _ranks(dsts)
    c_e = dsts // OWN
    rk = rank[dsts]
    w_e = rk // W                       # window
    g_e = w_e // GW                     # scatter group
    wl_e = w_e % GW                     # window within group
    p_e = 2 * (rk % W) + (r % 2)        # stream partition (pair slot)
    col_e = gbase[g_e] + (r // 2) * (nw_g[g_e] * F) + wl_e * F

    xs = x * dinv[:, None]
    vals = xs[srcs] * dinv[dsts][:, None]           # [E+N, F] fp32
    totcols = int(gbase[-1])
    stream = np.zeros((NC, 128, totcols), dtype=np.float32)
    stream[c_e[:, None], p_e[:, None],
           col_e[:, None] + np.arange(F)[None, :]] = vals
    stream = _to_bf16(stream)
    spec = tuple(zip(map(int, nw_g), map(int, B_g)))
    return stream, spec, orders


def _build_program(spec, npos):
    import concourse.bass as bass
    import concourse.mybir as mybir
    from concourse import bacc
    from concourse.tile import TileContext

    totcols = sum(nw * bg * F for nw, bg in spec)
    sbtmax = max(bg * nw * F for nw, bg in spec)

    nc = bacc.Bacc("TRN2", target_bir_lowering=False, debug=False,
                   num_devices=NC)
    dt = mybir.dt

    stream = nc.dram_tensor("stream", [128, totcols], dt.bfloat16,
                            kind="ExternalInput")
    # consts blob: [0:64]=pair, [64:128]=ident, [128:640]=w1b —
    # one DMA instead of several
    consts = nc.dram_tensor("consts", [128, 640], dt.bfloat16,
                            kind="ExternalInput")
    yout = nc.dram_tensor("yout", [W, NCHUNK * CW], dt.bfloat16,
                          kind="ExternalOutput")

    with TileContext(nc) as tc:
        with (
            tc.tile_pool(name="persist", bufs=1) as pp,
            tc.tile_pool(name="stream", bufs=16) as sp,
            tc.tile_pool(name="work", bufs=4) as wp,
            tc.tile_pool(name="psum", bufs=2, space="PSUM") as psp,
            tc.tile_pool(name="psum_t", bufs=3, space="PSUM") as pst,
        ):
            cb = pp.tile([128, 640], dt.bfloat16)
            nc.scalar.dma_start(cb[:], consts[:])
            pair_sb = cb[:, 0:W]
            id_sb = cb[0:W, W:2 * W]
            w1b_sb = cb[:, 128:128 + CW * H]
            y_all = pp.tile([W, NCHUNK * CW], dt.bfloat16)
            # transposed-agg staging tiles; row 112 = constant 1.0
            # (multiplies the b1 row of the block-diagonal W1)
            NSTG = 8
            aggts = [pp.tile([128, W], dt.bfloat16, name=f"aggts{i}")
                     for i in range(NSTG)]
            for t in aggts:
                nc.vector.memset(t[:], 1.0)

            def tail_a(g, nw, pgrp):
                nchu = -(-nw // CW)
                # phase 1: psum->sbuf copies (paired chunks, fewer ACT
                # fixed costs) + PE transposes
                stage = []
                agg_sb = None
                for lc in range(nchu):
                    c = g * (GW // CW) + lc
                    if lc % 2 == 0:
                        wid = min(2, nchu - lc) * CW * F
                        agg_sb = wp.tile([W, 2 * CW * F], dt.bfloat16,
                                         tag="agg")
                        nc.scalar.activation(
                            agg_sb[:, 0:wid],
                            pgrp[0:W, lc * CW * F:lc * CW * F + wid],
                            mybir.ActivationFunctionType.Copy,
                        )
                    asl = agg_sb[:, (lc % 2) * CW * F:(lc % 2 + 1) * CW * F]
                    aggt_ps = pst.tile([CW * F, W], dt.bfloat16, tag="aggt")
                    nc.tensor.transpose(aggt_ps[:], asl, id_sb[:])
                    aggt = aggts[c % NSTG]
                    nc.scalar.activation(
                        aggt[0:CW * F, :], aggt_ps[:],
                        mybir.ActivationFunctionType.Copy,
                    )
                    stage.append((c, aggt))
                return stage

            def tail_b(stage):
                # phase 2: W1-with-b1 matmul, relu, W2 sign-split reduce
                for c, aggt in stage:
                    ph = pst.tile([H, CW * H], dt.float32, tag="ph")
                    nc.tensor.matmul(
                        out=ph[:], lhsT=aggt[0:CW * F + 1, :],
                        rhs=w1b_sb[0:CW * F + 1, :],
                        start=True, stop=True,
                    )
                    hb = wp.tile([H, CW * H], dt.bfloat16, tag="hb")
                    nc.scalar.activation(
                        hb[:], ph[:], mybir.ActivationFunctionType.Relu,
                    )
                    # |W2| is folded into w1b pre-relu; h columns are
                    # ordered positives-then-negatives, so y is the
                    # difference of two strided segment reduces
                    h3 = hb[:].rearrange("p (w h) -> p w h", h=H)
                    if 0 < npos < H:
                        yp = wp.tile([H, CW], dt.float32, tag="yp")
                        nc.vector.tensor_reduce(
                            out=yp[:], in_=h3[:, :, 0:npos],
                            axis=mybir.AxisListType.X,
                            op=mybir.AluOpType.add,
                        )
                        yn = wp.tile([H, CW], dt.float32, tag="yn")
                        nc.vector.tensor_reduce(
                            out=yn[:], in_=h3[:, :, npos:H],
                            axis=mybir.AxisListType.X,
                            op=mybir.AluOpType.add,
                        )
                        with nc.allow_low_precision("y output is bf16"):
                            nc.vector.tensor_tensor(
                                out=y_all[:, c * CW:(c + 1) * CW],
                                in0=yp[:], in1=yn[:],
                                op=mybir.AluOpType.subtract,
                            )
                    else:
                        yp = wp.tile([H, CW], dt.float32, tag="yp")
                        nc.vector.tensor_reduce(
                            out=yp[:], in_=h3[:],
                            axis=mybir.AxisListType.X,
                            op=mybir.AluOpType.add,
                        )
                        with nc.allow_low_precision("y output is bf16"):
                            nc.vector.tensor_scalar(
                                out=y_all[:, c * CW:(c + 1) * CW],
                                in0=yp[:],
                                scalar1=1.0 if npos == H else -1.0,
                                scalar2=None,
                                op0=mybir.AluOpType.mult,
                            )

            pending = []
            pending_b = []
            for g, (nw, bg) in enumerate(spec):
                off = sum(n_ * b_ * F for n_, b_ in spec[:g])
                # one tile per DMA chunk so matmuls only wait for the
                # chunk that covers their batch (dep granularity is
                # per tile, not per slice)
                step = 4
                bounds = list(range(1 if g == 0 else step, bg, step))
                bounds = [0] + bounds + [bg]
                chunks = []
                chunk_of = {}
                for ci in range(len(bounds) - 1):
                    k, hi = bounds[ci], bounds[ci + 1]
                    ct = sp.tile([128, step * GW * F], dt.bfloat16,
                                 tag="sbt")
                    nc.sync.dma_start(
                        ct[:, 0:(hi - k) * nw * F],
                        stream[:, off + k * nw * F:off + hi * nw * F],
                    )
                    chunks.append(ct)
                    for b in range(k, hi):
                        chunk_of[b] = (ct, b - k)
                pgrp = psp.tile([W, GW * F], dt.float32)
                for b in range(bg):
                    ct, lb = chunk_of[b]
                    nc.tensor.matmul(
                        out=pgrp[0:W, 0:nw * F],
                        lhsT=pair_sb[:],
                        rhs=ct[:, lb * nw * F:(lb + 1) * nw * F],
                        start=(b == 0), stop=(b == bg - 1),
                    )
                # phase 2 lags the scatter by two groups so its PE ops
                # enter the FIFO well after their ACT deps started
                if pending_b:
                    tail_b(pending_b.pop(0))
                pending.append((g, nw, pgrp))
                if len(pending) > 1:
                    pending_b.append(tail_a(*pending.pop(0)))
                if g == len(spec) - 1:
                    # first part of y is final once earlier tails ran
                    nc.sync.dma_start(yout[:, 0:96], y_all[:, 0:96])
            for p in pending:
                pending_b.append(tail_a(*p))
            for s in pending_b:
                tail_b(s)
            nc.sync.dma_start(yout[:, 96:], y_all[:, 96:])

    nc.compile()
    return nc


_CACHE = {}


def kernel(x, edge_index, W1, b1, W2, b2, _want_results_obj=False):
    from concourse import bass_utils

    x = np.asarray(x, dtype=np.float32)
    edge_index = np.asarray(edge_index)
    stream, spec, orders = _host_pack(x, edge_index)

    W1 = np.asarray(W1, dtype=np.float32)
    b1 = np.asarray(b1, dtype=np.float32).reshape(H)
    W2 = np.asarray(W2, dtype=np.float32).reshape(H)
    b2 = float(np.asarray(b2, dtype=np.float32).reshape(()))
    # fold |W2| into W1/b1 (pre-relu scaling); order h columns so
    # positive-W2 entries come first: y = sum(pos) - sum(neg)
    perm = np.argsort(W2 < 0, kind="stable")
    npos = int((W2 >= 0).sum())
    w1s = W1[:, perm] * np.abs(W2[perm])[None, :]
    b1s = b1[perm] * np.abs(W2[perm])

    key = (spec, npos)
    if key not in _CACHE:
        _CACHE[key] = _build_program(spec, npos)
    nc = _CACHE[key]

    consts = np.zeros((128, 640), dtype=np.float32)
    consts[:, 0:W] = np.repeat(np.eye(W, dtype=np.float32), 2, axis=0)
    consts[0:W, W:2 * W] = np.eye(W, dtype=np.float32)
    for w in range(CW):
        consts[w * F:(w + 1) * F, 128 + w * H:128 + (w + 1) * H] = w1s
        consts[CW * F, 128 + w * H:128 + (w + 1) * H] = b1s
    consts = _to_bf16(consts)

    in_maps = []
    for c in range(NC):
        in_maps.append({
            "stream": np.ascontiguousarray(stream[c]),
            "consts": consts,
        })

    try:
        res = bass_utils.run_bass_kernel_spmd(
            nc, in_maps, core_ids=list(range(NC)))
    except Exception:
        # transient NRT device faults recover on re-execution
        res = bass_utils.run_bass_kernel_spmd(
            nc, in_maps, core_ids=list(range(NC)))
    out = np.empty((N, 1), dtype=np.float32)
    for c in range(NC):
        y = np.asarray(res.results[c]["yout"], dtype=np.float32)
        # rank = win*64 + slot  ->  value y[slot, win]
        yr = y[:, :NWIN].T.reshape(-1)[:OWN]  # [win, slot] flat = rank
        out[c * OWN + orders[c], 0] = yr + b2
    if _want_results_obj:
        return out, res
    return out


def _to_bf16(a):
    """fp32 ndarray -> bfloat16 (round-to-nearest-even) as ml_dtypes array."""
    import ml_dtypes

    return a.astype(ml_dtypes.bfloat16)
